# revision 3
# baseline (speedup 1.0000x reference)
"""Trainium2 Bass kernel for nn_EmbeddingLayer (embedding lookup + dense + positional encoding).

Computation (per reference):
    x = emb_table[tokens] * sqrt(512)          [B, F, E]
    x = x.reshape(B, F*E) @ W + b              [B, M]
    out = x[:, None, :] + pe[:128, :]          [B, S, M]   (1 GiB f32 output)

Strategy: data-parallel over batch across 8 cores (512 rows each); the
74 MB table and the 4 MB dense weight are replicated.  Per core:
  - indirect-DMA gather, one token column per instruction (HW honors one
    index per partition), 256 gathers of 128 rows x 128 B
  - PE transpose -> xT, f32 matmul against pre-scaled W (scale and bias
    folded on host into W / pe)
  - pe rows broadcast across partitions with a ones(1x128) matmul on PE,
    copied psum->SBUF (bf16) on ScalarE
  - DVE broadcast-add y[b,m] + pe[s,m] (bf16 operands, f32 result)
  - 2 MiB HWDGE stores of the 128 MiB per-core output (the roofline)
Loop order streams per batch-chunk so adds/stores begin as soon as the
first chunk's gathers land instead of waiting for all gathers.
"""

import sys

import numpy as np

if "/opt/trn_rl_repo" not in sys.path:
    sys.path.insert(0, "/opt/trn_rl_repo")

BATCH = 4096
FEATURES = 64
VOCAB = 580000
EMB = 32
MODELS = 512
SEQ = 128
N_CORES = 8
BS = BATCH // N_CORES  # 512 rows per core

P = 128                 # partitions
NB = BS // P            # 4 batch chunks per core
KC = (FEATURES * EMB) // P  # 16 k-chunks of 128
G = 8                   # seq positions per output tile
NG = SEQ // G           # 16 seq groups

ADD_DT = "bf16"         # dtype of the add operands (y, pe bcast); out stays f32

_MODULE_CACHE = {}


def _positional_encoding(position, d_model):
    # mirror of reference._positional_encoding, in numpy f32
    pos = np.arange(position, dtype=np.float32)[:, None]
    i = np.arange(d_model, dtype=np.float32)[None, :]
    angle_rates = 1.0 / np.power(
        10000.0, (2.0 * np.floor(i / 2.0)) / np.float32(d_model)
    )
    angles = (pos * angle_rates).astype(np.float32)
    even = (np.arange(d_model) % 2 == 0)[None, :]
    pe = np.where(even, np.sin(angles), np.cos(angles)).astype(np.float32)
    return pe  # [S, M]


def build_module():
    """Build + compile the per-core Bass module (identical program on all cores)."""
    if "nc" in _MODULE_CACHE:
        return _MODULE_CACHE["nc"]

    from contextlib import ExitStack

    import concourse.bass as bass
    import concourse.tile as tile
    from concourse import bacc, mybir

    f32 = mybir.dt.float32
    i32 = mybir.dt.int32
    adt = mybir.dt.bfloat16 if ADD_DT == "bf16" else mybir.dt.float32

    nc = bacc.Bacc("TRN2", target_bir_lowering=False, debug=False,
                   num_devices=N_CORES)

    tok = nc.dram_tensor("tok", [BS, FEATURES], i32, kind="ExternalInput").ap()
    emb = nc.dram_tensor("emb", [VOCAB, EMB], f32, kind="ExternalInput").ap()
    w = nc.dram_tensor("w", [FEATURES * EMB, MODELS], f32, kind="ExternalInput").ap()
    pe = nc.dram_tensor("pe", [SEQ, MODELS], f32, kind="ExternalInput").ap()
    ident = nc.dram_tensor("ident", [P, P], f32, kind="ExternalInput").ap()
    ones = nc.dram_tensor("ones", [1, P], f32, kind="ExternalInput").ap()
    out = nc.dram_tensor("out", [BS, SEQ, MODELS], f32, kind="ExternalOutput").ap()

    with tile.TileContext(nc) as tc, ExitStack() as ctx:
        const = ctx.enter_context(tc.tile_pool(name="const", bufs=1))
        tok_pool = ctx.enter_context(tc.tile_pool(name="tok", bufs=NB))
        x_pool = ctx.enter_context(tc.tile_pool(name="x", bufs=2))
        xT_pool = ctx.enter_context(tc.tile_pool(name="xT", bufs=2))
        y_pool = ctx.enter_context(tc.tile_pool(name="y", bufs=NB))
        perow_pool = ctx.enter_context(tc.tile_pool(name="perow", bufs=2))
        pegrp_pool = ctx.enter_context(tc.tile_pool(name="pegrp", bufs=2))
        out_pool = ctx.enter_context(tc.tile_pool(name="outp", bufs=4))
        psum_t = ctx.enter_context(tc.tile_pool(name="pst", bufs=2, space="PSUM"))
        psum_y = ctx.enter_context(tc.tile_pool(name="psy", bufs=2, space="PSUM"))
        psum_p = ctx.enter_context(tc.tile_pool(name="psp", bufs=2, space="PSUM"))

        # constants
        w_sb = const.tile([P, KC * MODELS], f32)
        nc.sync.dma_start(
            w_sb[:].rearrange("p (kc m) -> p kc m", kc=KC),
            w.rearrange("(kc p) m -> p kc m", p=P),
        )
        id_sb = const.tile([P, P], f32)
        nc.sync.dma_start(id_sb[:], ident[:])
        ones_sb = const.tile([1, P], f32)
        nc.sync.dma_start(ones_sb[:], ones[:])

        tok_sbs = []
        for c in range(NB):
            tok_sb = tok_pool.tile([P, FEATURES], i32)
            nc.sync.dma_start(tok_sb[:], tok[c * P:(c + 1) * P, :])
            tok_sbs.append(tok_sb)

        for c in range(NB):
            tok_sb = tok_sbs[c]
            # gather one token column per indirect DMA (128 rows x 128 B),
            # interleaved with the transposes that consume them
            x_sb = x_pool.tile([P, FEATURES * EMB], f32)
            xT_sb = xT_pool.tile([P, KC * P], f32)
            for kc in range(KC):
                for j in range(4):
                    f = 4 * kc + j
                    nc.gpsimd.indirect_dma_start(
                        out=x_sb[:, f * EMB:(f + 1) * EMB],
                        out_offset=None,
                        in_=emb[:],
                        in_offset=bass.IndirectOffsetOnAxis(
                            ap=tok_sb[:, f:f + 1], axis=0
                        ),
                    )
                pt = psum_t.tile([P, P], f32)
                nc.tensor.transpose(
                    out=pt[:], in_=x_sb[:, kc * P:(kc + 1) * P], identity=id_sb[:]
                )
                nc.vector.tensor_copy(xT_sb[:, kc * P:(kc + 1) * P], pt[:])

            py = psum_y.tile([P, MODELS], f32)
            for kc in range(KC):
                nc.tensor.matmul(
                    py[:],
                    lhsT=xT_sb[:, kc * P:(kc + 1) * P],
                    rhs=w_sb[:, kc * MODELS:(kc + 1) * MODELS],
                    start=(kc == 0),
                    stop=(kc == KC - 1),
                )
            y_sb = y_pool.tile([P, MODELS], adt)
            nc.scalar.copy(y_sb[:], py[:])

            # stream this chunk's output: for each seq group, broadcast pe
            # rows across partitions (ones-matmul), add, store
            for g in range(NG):
                perow = perow_pool.tile([1, G * MODELS], f32)
                nc.sync.dma_start(
                    perow[:].rearrange("p (g m) -> p g m", g=G),
                    pe[g * G:(g + 1) * G, :].unsqueeze(0),
                )
                peg = pegrp_pool.tile([P, G * MODELS], adt)
                for sl in range(G):
                    pp = psum_p.tile([P, MODELS], f32)
                    nc.tensor.matmul(
                        pp[:],
                        lhsT=ones_sb[:],
                        rhs=perow[:, sl * MODELS:(sl + 1) * MODELS],
                        start=True,
                        stop=True,
                    )
                    nc.scalar.copy(peg[:, sl * MODELS:(sl + 1) * MODELS], pp[:])

                ot = out_pool.tile([P, G * MODELS], f32)
                nc.vector.tensor_tensor(
                    out=ot[:].rearrange("p (g m) -> p g m", g=G),
                    in0=y_sb[:].unsqueeze(1).to_broadcast([P, G, MODELS]),
                    in1=peg[:].rearrange("p (g m) -> p g m", g=G),
                    op=mybir.AluOpType.add,
                )
                nc.sync.dma_start(
                    out[c * P:(c + 1) * P, g * G:(g + 1) * G, :],
                    ot[:].rearrange("p (g m) -> p g m", g=G),
                )

    nc.compile()
    _MODULE_CACHE["nc"] = nc
    return nc


def make_in_maps(tokens, emb_table, W, b):
    tokens = np.ascontiguousarray(np.asarray(tokens, dtype=np.int32))
    emb_table = np.ascontiguousarray(np.asarray(emb_table, dtype=np.float32))
    W = np.asarray(W, dtype=np.float32)
    b = np.asarray(b, dtype=np.float32)

    wp = np.ascontiguousarray(W * np.float32(np.sqrt(np.float32(MODELS))))
    peb = np.ascontiguousarray(
        _positional_encoding(SEQ, MODELS) + b[None, :].astype(np.float32)
    )
    ident = np.eye(P, dtype=np.float32)
    ones = np.ones((1, P), dtype=np.float32)

    in_maps = []
    for c in range(N_CORES):
        in_maps.append({
            "tok": tokens[c * BS:(c + 1) * BS],
            "emb": emb_table,
            "w": wp,
            "pe": peb,
            "ident": ident,
            "ones": ones,
        })
    return in_maps


def run(tokens, emb_table, W, b, trace=False):
    """Run on 8 NeuronCores; returns (full_output, BassKernelResults)."""
    from concourse import bass_utils

    nc = build_module()
    in_maps = make_in_maps(tokens, emb_table, W, b)
    res = bass_utils.run_bass_kernel_spmd(
        nc, in_maps, core_ids=list(range(N_CORES)), trace=trace
    )
    outs = [r["out"] for r in res.results]
    full = np.concatenate(outs, axis=0)
    return full, res


def kernel(tokens, emb_table, W, b):
    full, _ = run(tokens, emb_table, W, b, trace=False)
    return full


# revision 4
# speedup vs baseline: 1.3695x; 1.3695x over previous
"""Trainium2 Bass kernel for nn_EmbeddingLayer (embedding lookup + dense + positional encoding).

Computation (per reference):
    x = emb_table[tokens] * sqrt(512)          [B, F, E]
    x = x.reshape(B, F*E) @ W + b              [B, M]
    out = x[:, None, :] + pe[:128, :]          [B, S, M]   (1 GiB f32 output)

Strategy: data-parallel over batch across 8 cores (512 rows each); the
74 MB table and the dense weight are replicated.  Per core:
  - indirect-DMA gather, one token column per instruction (HW honors one
    index per partition), 256 gathers of 128 rows x 128 B
  - PE transpose (f32 fast transpose mode) -> xT cast to bf16, bf16
    matmul against pre-scaled bf16 W (scale/bias folded on host)
  - pe rows broadcast across partitions with a ones(1x128) bf16 matmul
    on PE; the f32 psum result is consumed directly by the DVE add
    (no psum->SBUF copy)
  - DVE broadcast-add y[b,m] + pe[s,m] -> f32 out tiles
  - 1 MiB HWDGE stores of the 128 MiB per-core output (the roofline)
Loop order streams per batch-chunk so adds/stores begin as soon as the
first chunk's gathers land.
"""

import sys

import numpy as np

if "/opt/trn_rl_repo" not in sys.path:
    sys.path.insert(0, "/opt/trn_rl_repo")

BATCH = 4096
FEATURES = 64
VOCAB = 580000
EMB = 32
MODELS = 512
SEQ = 128
N_CORES = 8
BS = BATCH // N_CORES  # 512 rows per core

P = 128                 # partitions
NB = BS // P            # 4 batch chunks per core
KC = (FEATURES * EMB) // P  # 16 k-chunks of 128

_MODULE_CACHE = {}


def _positional_encoding(position, d_model):
    # mirror of reference._positional_encoding, in numpy f32
    pos = np.arange(position, dtype=np.float32)[:, None]
    i = np.arange(d_model, dtype=np.float32)[None, :]
    angle_rates = 1.0 / np.power(
        10000.0, (2.0 * np.floor(i / 2.0)) / np.float32(d_model)
    )
    angles = (pos * angle_rates).astype(np.float32)
    even = (np.arange(d_model) % 2 == 0)[None, :]
    pe = np.where(even, np.sin(angles), np.cos(angles)).astype(np.float32)
    return pe  # [S, M]


def build_module():
    """Build + compile the per-core Bass module (identical program on all cores)."""
    if "nc" in _MODULE_CACHE:
        return _MODULE_CACHE["nc"]

    from contextlib import ExitStack

    import concourse.bass as bass
    import concourse.tile as tile
    from concourse import bacc, mybir

    f32 = mybir.dt.float32
    bf16 = mybir.dt.bfloat16
    i32 = mybir.dt.int32

    nc = bacc.Bacc("TRN2", target_bir_lowering=False, debug=False,
                   num_devices=N_CORES)

    tok = nc.dram_tensor("tok", [BS, FEATURES], i32, kind="ExternalInput").ap()
    emb = nc.dram_tensor("emb", [VOCAB, EMB], f32, kind="ExternalInput").ap()
    w = nc.dram_tensor("w", [FEATURES * EMB, MODELS], bf16, kind="ExternalInput").ap()
    pe = nc.dram_tensor("pe", [SEQ, MODELS], bf16, kind="ExternalInput").ap()
    ident = nc.dram_tensor("ident", [P, P], f32, kind="ExternalInput").ap()
    ones = nc.dram_tensor("ones", [1, P], bf16, kind="ExternalInput").ap()
    out = nc.dram_tensor("out", [BS, SEQ, MODELS], f32, kind="ExternalOutput").ap()

    with tile.TileContext(nc) as tc, ExitStack() as ctx:
        const = ctx.enter_context(tc.tile_pool(name="const", bufs=1))
        tok_pool = ctx.enter_context(tc.tile_pool(name="tok", bufs=NB))
        x_pool = ctx.enter_context(tc.tile_pool(name="x", bufs=2))
        xT_pool = ctx.enter_context(tc.tile_pool(name="xT", bufs=2))
        y_pool = ctx.enter_context(tc.tile_pool(name="y", bufs=NB))
        perow_pool = ctx.enter_context(tc.tile_pool(name="perow", bufs=2))
        out_pool = ctx.enter_context(tc.tile_pool(name="outp", bufs=6))
        psum_t = ctx.enter_context(tc.tile_pool(name="pst", bufs=2, space="PSUM"))
        psum_y = ctx.enter_context(tc.tile_pool(name="psy", bufs=2, space="PSUM"))
        psum_p = ctx.enter_context(tc.tile_pool(name="psp", bufs=2, space="PSUM"))

        # constants
        w_sb = const.tile([P, KC * MODELS], bf16)
        nc.sync.dma_start(
            w_sb[:].rearrange("p (kc m) -> p kc m", kc=KC),
            w.rearrange("(kc p) m -> p kc m", p=P),
        )
        id_sb = const.tile([P, P], f32)
        nc.sync.dma_start(id_sb[:], ident[:])
        ones_sb = const.tile([1, P], bf16)
        nc.sync.dma_start(ones_sb[:], ones[:])

        tok_sbs = []
        for c in range(NB):
            tok_sb = tok_pool.tile([P, FEATURES], i32)
            nc.sync.dma_start(tok_sb[:], tok[c * P:(c + 1) * P, :])
            tok_sbs.append(tok_sb)

        for c in range(NB):
            tok_sb = tok_sbs[c]
            x_sb = x_pool.tile([P, FEATURES * EMB], f32)
            xT_sb = xT_pool.tile([P, KC * P], bf16)
            # gather one token column per indirect DMA, transpose per k-chunk,
            # batch 4 transposes per psum bank before the cast-copy to bf16
            for kq in range(4):
                pt = psum_t.tile([P, 4 * P], f32)
                for j in range(4):
                    kc = 4 * kq + j
                    for jf in range(4):
                        f = 4 * kc + jf
                        nc.gpsimd.indirect_dma_start(
                            out=x_sb[:, f * EMB:(f + 1) * EMB],
                            out_offset=None,
                            in_=emb[:],
                            in_offset=bass.IndirectOffsetOnAxis(
                                ap=tok_sb[:, f:f + 1], axis=0
                            ),
                        )
                    nc.tensor.transpose(
                        out=pt[:, j * P:(j + 1) * P],
                        in_=x_sb[:, kc * P:(kc + 1) * P],
                        identity=id_sb[:],
                    )
                nc.vector.tensor_copy(xT_sb[:, kq * 4 * P:(kq + 1) * 4 * P], pt[:])

            py = psum_y.tile([P, MODELS], f32)
            for kc in range(KC):
                nc.tensor.matmul(
                    py[:],
                    lhsT=xT_sb[:, kc * P:(kc + 1) * P],
                    rhs=w_sb[:, kc * MODELS:(kc + 1) * MODELS],
                    start=(kc == 0),
                    stop=(kc == KC - 1),
                )
            y_sb = y_pool.tile([P, MODELS], f32)
            nc.scalar.copy(y_sb[:], py[:])

            # stream this chunk's output: pe rows -> psum via ones-matmul,
            # DVE adds psum + broadcast y, 1 MiB stores (4 seq rows per tile)
            for gq in range(16):
                perow = perow_pool.tile([1, 8 * MODELS], bf16)
                nc.sync.dma_start(
                    perow[:].rearrange("p (g m) -> p g m", g=8),
                    pe[gq * 8:(gq + 1) * 8, :].unsqueeze(0),
                )
                for t in range(2):
                    ot = out_pool.tile([P, 4 * MODELS], f32)
                    for h in range(2):
                        pp = psum_p.tile([P, 2 * MODELS], f32)
                        for u in range(2):
                            sl = t * 4 + h * 2 + u
                            nc.tensor.matmul(
                                pp[:, u * MODELS:(u + 1) * MODELS],
                                lhsT=ones_sb[:],
                                rhs=perow[:, sl * MODELS:(sl + 1) * MODELS],
                                start=True,
                                stop=True,
                            )
                        nc.vector.tensor_tensor(
                            out=ot[:, h * 2 * MODELS:(h + 1) * 2 * MODELS]
                                .rearrange("p (g m) -> p g m", g=2),
                            in0=y_sb[:].unsqueeze(1).to_broadcast([P, 2, MODELS]),
                            in1=pp[:].rearrange("p (g m) -> p g m", g=2),
                            op=mybir.AluOpType.add,
                        )
                    s0 = gq * 8 + t * 4
                    nc.sync.dma_start(
                        out[c * P:(c + 1) * P, s0:s0 + 4, :],
                        ot[:].rearrange("p (g m) -> p g m", g=4),
                    )

    nc.compile()
    _MODULE_CACHE["nc"] = nc
    return nc


def make_in_maps(tokens, emb_table, W, b):
    import ml_dtypes

    tokens = np.ascontiguousarray(np.asarray(tokens, dtype=np.int32))
    emb_table = np.ascontiguousarray(np.asarray(emb_table, dtype=np.float32))
    W = np.asarray(W, dtype=np.float32)
    b = np.asarray(b, dtype=np.float32)

    wp = np.ascontiguousarray(
        (W * np.float32(np.sqrt(np.float32(MODELS)))).astype(ml_dtypes.bfloat16)
    )
    peb = np.ascontiguousarray(
        (_positional_encoding(SEQ, MODELS) + b[None, :].astype(np.float32))
        .astype(ml_dtypes.bfloat16)
    )
    ident = np.eye(P, dtype=np.float32)
    ones = np.ones((1, P), dtype=ml_dtypes.bfloat16)

    in_maps = []
    for c in range(N_CORES):
        in_maps.append({
            "tok": tokens[c * BS:(c + 1) * BS],
            "emb": emb_table,
            "w": wp,
            "pe": peb,
            "ident": ident,
            "ones": ones,
        })
    return in_maps


def run(tokens, emb_table, W, b, trace=False):
    """Run on 8 NeuronCores; returns (full_output, BassKernelResults)."""
    from concourse import bass_utils

    nc = build_module()
    in_maps = make_in_maps(tokens, emb_table, W, b)
    res = bass_utils.run_bass_kernel_spmd(
        nc, in_maps, core_ids=list(range(N_CORES)), trace=trace
    )
    outs = [r["out"] for r in res.results]
    full = np.concatenate(outs, axis=0)
    return full, res


def kernel(tokens, emb_table, W, b):
    full, _ = run(tokens, emb_table, W, b, trace=False)
    return full


# revision 6
# speedup vs baseline: 1.6776x; 1.2250x over previous
"""Trainium2 Bass kernel for nn_EmbeddingLayer (embedding lookup + dense + positional encoding).

Computation (per reference):
    x = emb_table[tokens] * sqrt(512)          [B, F, E]
    x = x.reshape(B, F*E) @ W + b              [B, M]
    out = x[:, None, :] + pe[:128, :]          [B, S, M]   (1 GiB f32 output)

Strategy: data-parallel over batch across 8 cores (512 rows each); the
74 MB table and the dense weight are replicated.  Per core:
  - indirect-DMA gather, one token column per instruction (HW honors one
    index per partition), 256 gathers of 128 rows x 128 B
  - PE transpose (f32 fast transpose mode) -> xT cast to bf16, bf16
    matmul against pre-scaled bf16 W (scale/bias folded on host)
  - pe rows broadcast across partitions with a ones(1x128) bf16 matmul
    on PE; the f32 psum result is consumed directly by the DVE add
    (no psum->SBUF copy)
  - DVE broadcast-add y[b,m] + pe[s,m] -> f32 out tiles
  - 1 MiB HWDGE stores of the 128 MiB per-core output (the roofline)
Loop order streams per batch-chunk so adds/stores begin as soon as the
first chunk's gathers land.
"""

import sys

import numpy as np

if "/opt/trn_rl_repo" not in sys.path:
    sys.path.insert(0, "/opt/trn_rl_repo")

BATCH = 4096
FEATURES = 64
VOCAB = 580000
EMB = 32
MODELS = 512
SEQ = 128
N_CORES = 8
BS = BATCH // N_CORES  # 512 rows per core

P = 128                 # partitions
NB = BS // P            # 4 batch chunks per core
KC = (FEATURES * EMB) // P  # 16 k-chunks of 128

_MODULE_CACHE = {}


def _positional_encoding(position, d_model):
    # mirror of reference._positional_encoding, in numpy f32
    pos = np.arange(position, dtype=np.float32)[:, None]
    i = np.arange(d_model, dtype=np.float32)[None, :]
    angle_rates = 1.0 / np.power(
        10000.0, (2.0 * np.floor(i / 2.0)) / np.float32(d_model)
    )
    angles = (pos * angle_rates).astype(np.float32)
    even = (np.arange(d_model) % 2 == 0)[None, :]
    pe = np.where(even, np.sin(angles), np.cos(angles)).astype(np.float32)
    return pe  # [S, M]


def build_module():
    """Build + compile the per-core Bass module (identical program on all cores)."""
    if "nc" in _MODULE_CACHE:
        return _MODULE_CACHE["nc"]

    from contextlib import ExitStack

    import concourse.bass as bass
    import concourse.tile as tile
    from concourse import bacc, mybir

    f32 = mybir.dt.float32
    bf16 = mybir.dt.bfloat16
    i32 = mybir.dt.int32

    nc = bacc.Bacc("TRN2", target_bir_lowering=False, debug=False,
                   num_devices=N_CORES)

    tok = nc.dram_tensor("tok", [BS, FEATURES], i32, kind="ExternalInput").ap()
    emb = nc.dram_tensor("emb", [VOCAB, EMB], f32, kind="ExternalInput").ap()
    w = nc.dram_tensor("w", [FEATURES * EMB, MODELS], bf16, kind="ExternalInput").ap()
    pe = nc.dram_tensor("pe", [SEQ, MODELS], bf16, kind="ExternalInput").ap()
    ident = nc.dram_tensor("ident", [P, P], f32, kind="ExternalInput").ap()
    ones = nc.dram_tensor("ones", [1, P], bf16, kind="ExternalInput").ap()
    out = nc.dram_tensor("out", [BS, SEQ, MODELS], f32, kind="ExternalOutput").ap()

    with tile.TileContext(nc) as tc, ExitStack() as ctx:
        const = ctx.enter_context(tc.tile_pool(name="const", bufs=1))
        tok_pool = ctx.enter_context(tc.tile_pool(name="tok", bufs=NB))
        x_pool = ctx.enter_context(tc.tile_pool(name="x", bufs=2))
        xT_pool = ctx.enter_context(tc.tile_pool(name="xT", bufs=2))
        y_pool = ctx.enter_context(tc.tile_pool(name="y", bufs=NB))
        perow_pool = ctx.enter_context(tc.tile_pool(name="perow", bufs=4))
        out_pool = ctx.enter_context(tc.tile_pool(name="outp", bufs=3))
        psum_t = ctx.enter_context(tc.tile_pool(name="pst", bufs=2, space="PSUM"))
        psum_y = ctx.enter_context(tc.tile_pool(name="psy", bufs=2, space="PSUM"))
        psum_p = ctx.enter_context(tc.tile_pool(name="psp", bufs=2, space="PSUM"))

        # constants — all input loads go on the ScalarE HWDGE ring so the
        # Sync ring carries only the big output stores
        w_sb = const.tile([P, KC * MODELS], bf16)
        nc.scalar.dma_start(
            w_sb[:].rearrange("p (kc m) -> p kc m", kc=KC),
            w.rearrange("(kc p) m -> p kc m", p=P),
        )
        id_sb = const.tile([P, P], f32)
        nc.scalar.dma_start(id_sb[:], ident[:])
        ones_sb = const.tile([1, P], bf16)
        nc.scalar.dma_start(ones_sb[:], ones[:])

        tok_sbs = []
        for c in range(NB):
            tok_sb = tok_pool.tile([P, FEATURES], i32)
            nc.scalar.dma_start(tok_sb[:], tok[c * P:(c + 1) * P, :])
            tok_sbs.append(tok_sb)

        for c in range(NB):
            tok_sb = tok_sbs[c]
            x_sb = x_pool.tile([P, FEATURES * EMB], f32)
            xT_sb = xT_pool.tile([P, KC * P], bf16)
            # gather one token column per indirect DMA, transpose per k-chunk,
            # batch 4 transposes per psum bank before the cast-copy to bf16
            for kq in range(4):
                pt = psum_t.tile([P, 4 * P], f32)
                for j in range(4):
                    kc = 4 * kq + j
                    for jf in range(4):
                        f = 4 * kc + jf
                        nc.gpsimd.indirect_dma_start(
                            out=x_sb[:, f * EMB:(f + 1) * EMB],
                            out_offset=None,
                            in_=emb[:],
                            in_offset=bass.IndirectOffsetOnAxis(
                                ap=tok_sb[:, f:f + 1], axis=0
                            ),
                        )
                    nc.tensor.transpose(
                        out=pt[:, j * P:(j + 1) * P],
                        in_=x_sb[:, kc * P:(kc + 1) * P],
                        identity=id_sb[:],
                    )
                nc.vector.tensor_copy(xT_sb[:, kq * 4 * P:(kq + 1) * 4 * P], pt[:])

            py = psum_y.tile([P, MODELS], f32)
            for kc in range(KC):
                nc.tensor.matmul(
                    py[:],
                    lhsT=xT_sb[:, kc * P:(kc + 1) * P],
                    rhs=w_sb[:, kc * MODELS:(kc + 1) * MODELS],
                    start=(kc == 0),
                    stop=(kc == KC - 1),
                )
            y_sb = y_pool.tile([P, MODELS], f32)
            nc.scalar.copy(y_sb[:], py[:])

            # stream this chunk's output: pe rows -> psum via ones-matmul,
            # DVE adds psum + broadcast y, 4 MiB stores (16 seq rows per tile)
            for go in range(8):
                ot = out_pool.tile([P, 16 * MODELS], f32)
                for half in range(2):
                    gq = go * 2 + half
                    perow = perow_pool.tile([1, 8 * MODELS], bf16)
                    nc.scalar.dma_start(
                        perow[:].rearrange("p (g m) -> p g m", g=8),
                        pe[gq * 8:(gq + 1) * 8, :].unsqueeze(0),
                    )
                    for h in range(4):
                        pp = psum_p.tile([P, 2 * MODELS], f32)
                        for u in range(2):
                            sl = h * 2 + u
                            nc.tensor.matmul(
                                pp[:, u * MODELS:(u + 1) * MODELS],
                                lhsT=ones_sb[:],
                                rhs=perow[:, sl * MODELS:(sl + 1) * MODELS],
                                start=True,
                                stop=True,
                            )
                        o0 = (half * 8 + h * 2) * MODELS
                        nc.vector.tensor_tensor(
                            out=ot[:, o0:o0 + 2 * MODELS]
                                .rearrange("p (g m) -> p g m", g=2),
                            in0=y_sb[:].unsqueeze(1).to_broadcast([P, 2, MODELS]),
                            in1=pp[:].rearrange("p (g m) -> p g m", g=2),
                            op=mybir.AluOpType.add,
                        )
                s0 = go * 16
                nc.sync.dma_start(
                    out[c * P:(c + 1) * P, s0:s0 + 16, :],
                    ot[:].rearrange("p (g m) -> p g m", g=16),
                )

    nc.compile()
    _MODULE_CACHE["nc"] = nc
    return nc


def make_in_maps(tokens, emb_table, W, b):
    import ml_dtypes

    tokens = np.ascontiguousarray(np.asarray(tokens, dtype=np.int32))
    emb_table = np.ascontiguousarray(np.asarray(emb_table, dtype=np.float32))
    W = np.asarray(W, dtype=np.float32)
    b = np.asarray(b, dtype=np.float32)

    wp = np.ascontiguousarray(
        (W * np.float32(np.sqrt(np.float32(MODELS)))).astype(ml_dtypes.bfloat16)
    )
    peb = np.ascontiguousarray(
        (_positional_encoding(SEQ, MODELS) + b[None, :].astype(np.float32))
        .astype(ml_dtypes.bfloat16)
    )
    ident = np.eye(P, dtype=np.float32)
    ones = np.ones((1, P), dtype=ml_dtypes.bfloat16)

    in_maps = []
    for c in range(N_CORES):
        in_maps.append({
            "tok": tokens[c * BS:(c + 1) * BS],
            "emb": emb_table,
            "w": wp,
            "pe": peb,
            "ident": ident,
            "ones": ones,
        })
    return in_maps


def run(tokens, emb_table, W, b, trace=False):
    """Run on 8 NeuronCores; returns (full_output, BassKernelResults)."""
    from concourse import bass_utils

    nc = build_module()
    in_maps = make_in_maps(tokens, emb_table, W, b)
    res = bass_utils.run_bass_kernel_spmd(
        nc, in_maps, core_ids=list(range(N_CORES)), trace=trace
    )
    outs = [r["out"] for r in res.results]
    full = np.concatenate(outs, axis=0)
    return full, res


def kernel(tokens, emb_table, W, b):
    full, _ = run(tokens, emb_table, W, b, trace=False)
    return full


# revision 8
# speedup vs baseline: 1.6864x; 1.0053x over previous
"""Trainium2 Bass kernel for nn_EmbeddingLayer (embedding lookup + dense + positional encoding).

Computation (per reference):
    x = emb_table[tokens] * sqrt(512)          [B, F, E]
    x = x.reshape(B, F*E) @ W + b              [B, M]
    out = x[:, None, :] + pe[:128, :]          [B, S, M]   (1 GiB f32 output)

Strategy: data-parallel over batch across 8 cores (512 rows each); the
74 MB table and the dense weight are replicated.  Per core:
  - indirect-DMA gather, one token column per instruction (HW honors one
    index per partition), 256 gathers of 128 rows x 128 B
  - PE transpose (f32 fast transpose mode) -> xT cast to bf16, bf16
    matmul against pre-scaled bf16 W (scale/bias folded on host)
  - pe rows broadcast across partitions with a ones(1x128) bf16 matmul
    on PE; the f32 psum result is consumed directly by the DVE add
    (no psum->SBUF copy)
  - DVE broadcast-add y[b,m] + pe[s,m] -> f32 out tiles
  - 1 MiB HWDGE stores of the 128 MiB per-core output (the roofline)
Loop order streams per batch-chunk so adds/stores begin as soon as the
first chunk's gathers land.
"""

import sys

import numpy as np

if "/opt/trn_rl_repo" not in sys.path:
    sys.path.insert(0, "/opt/trn_rl_repo")

BATCH = 4096
FEATURES = 64
VOCAB = 580000
EMB = 32
MODELS = 512
SEQ = 128
N_CORES = 8
BS = BATCH // N_CORES  # 512 rows per core

P = 128                 # partitions
NB = BS // P            # 4 batch chunks per core
KC = (FEATURES * EMB) // P  # 16 k-chunks of 128

_MODULE_CACHE = {}


def _positional_encoding(position, d_model):
    # mirror of reference._positional_encoding, in numpy f32
    pos = np.arange(position, dtype=np.float32)[:, None]
    i = np.arange(d_model, dtype=np.float32)[None, :]
    angle_rates = 1.0 / np.power(
        10000.0, (2.0 * np.floor(i / 2.0)) / np.float32(d_model)
    )
    angles = (pos * angle_rates).astype(np.float32)
    even = (np.arange(d_model) % 2 == 0)[None, :]
    pe = np.where(even, np.sin(angles), np.cos(angles)).astype(np.float32)
    return pe  # [S, M]


def build_module():
    """Build + compile the per-core Bass module (identical program on all cores)."""
    if "nc" in _MODULE_CACHE:
        return _MODULE_CACHE["nc"]

    from contextlib import ExitStack

    import concourse.bass as bass
    import concourse.tile as tile
    from concourse import bacc, mybir

    f32 = mybir.dt.float32
    bf16 = mybir.dt.bfloat16
    i32 = mybir.dt.int32

    nc = bacc.Bacc("TRN2", target_bir_lowering=False, debug=False,
                   num_devices=N_CORES)

    tok = nc.dram_tensor("tok", [BS, FEATURES], i32, kind="ExternalInput").ap()
    emb = nc.dram_tensor("emb", [VOCAB, EMB], f32, kind="ExternalInput").ap()
    w = nc.dram_tensor("w", [FEATURES * EMB, MODELS], bf16, kind="ExternalInput").ap()
    pe = nc.dram_tensor("pe", [SEQ, MODELS], bf16, kind="ExternalInput").ap()
    ident = nc.dram_tensor("ident", [P, P], f32, kind="ExternalInput").ap()
    ones = nc.dram_tensor("ones", [1, P], bf16, kind="ExternalInput").ap()
    out = nc.dram_tensor("out", [BS, SEQ, MODELS], f32, kind="ExternalOutput").ap()

    with tile.TileContext(nc) as tc, ExitStack() as ctx:
        const = ctx.enter_context(tc.tile_pool(name="const", bufs=1))
        tok_pool = ctx.enter_context(tc.tile_pool(name="tok", bufs=NB))
        x_pool = ctx.enter_context(tc.tile_pool(name="x", bufs=2))
        xT_pool = ctx.enter_context(tc.tile_pool(name="xT", bufs=2))
        y_pool = ctx.enter_context(tc.tile_pool(name="y", bufs=NB))
        perow_pool = ctx.enter_context(tc.tile_pool(name="perow", bufs=8))
        out_pool = ctx.enter_context(tc.tile_pool(name="outp", bufs=3))
        psum_t = ctx.enter_context(tc.tile_pool(name="pst", bufs=2, space="PSUM"))
        psum_y = ctx.enter_context(tc.tile_pool(name="psy", bufs=2, space="PSUM"))
        psum_p = ctx.enter_context(tc.tile_pool(name="psp", bufs=2, space="PSUM"))

        # constants — all input loads go on the ScalarE HWDGE ring so the
        # Sync ring carries only the big output stores
        w_sb = const.tile([P, KC * MODELS], bf16)
        nc.scalar.dma_start(
            w_sb[:].rearrange("p (kc m) -> p kc m", kc=KC),
            w.rearrange("(kc p) m -> p kc m", p=P),
        )
        id_sb = const.tile([P, P], f32)
        nc.scalar.dma_start(id_sb[:], ident[:])
        ones_sb = const.tile([1, P], bf16)
        nc.scalar.dma_start(ones_sb[:], ones[:])

        tok_sbs = []
        for c in range(NB):
            tok_sb = tok_pool.tile([P, FEATURES], i32)
            nc.scalar.dma_start(tok_sb[:], tok[c * P:(c + 1) * P, :])
            tok_sbs.append(tok_sb)

        # deferred output stores: chunk c's stores are interleaved into chunk
        # c+1's gather stream on the SAME SWDGE queue so the gathers can't
        # starve them (HWDGE queues get starved while SWDGE traffic flows)
        pending_stores = []

        def emit_store(c, go, ot):
            s0 = go * 16
            nc.gpsimd.dma_start(
                out[c * P:(c + 1) * P, s0:s0 + 16, :],
                ot[:].rearrange("p (g m) -> p g m", g=16),
            )

        for c in range(NB):
            tok_sb = tok_sbs[c]
            x_sb = x_pool.tile([P, FEATURES * EMB], f32)
            xT_sb = xT_pool.tile([P, KC * P], bf16)
            # gather one token column per indirect DMA, transpose per k-chunk,
            # batch 4 transposes per psum bank before the cast-copy to bf16
            for kq in range(4):
                pt = psum_t.tile([P, 4 * P], f32)
                for j in range(4):
                    kc = 4 * kq + j
                    for jf in range(4):
                        f = 4 * kc + jf
                        nc.gpsimd.indirect_dma_start(
                            out=x_sb[:, f * EMB:(f + 1) * EMB],
                            out_offset=None,
                            in_=emb[:],
                            in_offset=bass.IndirectOffsetOnAxis(
                                ap=tok_sb[:, f:f + 1], axis=0
                            ),
                        )
                    nc.tensor.transpose(
                        out=pt[:, j * P:(j + 1) * P],
                        in_=x_sb[:, kc * P:(kc + 1) * P],
                        identity=id_sb[:],
                    )
                nc.vector.tensor_copy(xT_sb[:, kq * 4 * P:(kq + 1) * 4 * P], pt[:])
                # two stores of the previous chunk ride along per k-quarter
                for _ in range(2):
                    if pending_stores:
                        emit_store(*pending_stores.pop(0))

            py = psum_y.tile([P, MODELS], f32)
            for kc in range(KC):
                nc.tensor.matmul(
                    py[:],
                    lhsT=xT_sb[:, kc * P:(kc + 1) * P],
                    rhs=w_sb[:, kc * MODELS:(kc + 1) * MODELS],
                    start=(kc == 0),
                    stop=(kc == KC - 1),
                )
            y_sb = y_pool.tile([P, MODELS], f32)
            nc.scalar.copy(y_sb[:], py[:])

            # stream this chunk's output: pe rows -> psum via ones-matmul,
            # DVE adds psum + broadcast y, 4 MiB stores (16 seq rows per tile)
            for go in range(8):
                ot = out_pool.tile([P, 16 * MODELS], f32)
                for quarter in range(4):
                    gq4 = go * 4 + quarter
                    perow = perow_pool.tile([1, 4 * MODELS], bf16)
                    nc.scalar.dma_start(
                        perow[:].rearrange("p (g m) -> p g m", g=4),
                        pe[gq4 * 4:(gq4 + 1) * 4, :].unsqueeze(0),
                    )
                    for h in range(2):
                        pp = psum_p.tile([P, 2 * MODELS], f32)
                        for u in range(2):
                            sl = h * 2 + u
                            nc.tensor.matmul(
                                pp[:, u * MODELS:(u + 1) * MODELS],
                                lhsT=ones_sb[:],
                                rhs=perow[:, sl * MODELS:(sl + 1) * MODELS],
                                start=True,
                                stop=True,
                            )
                        o0 = (quarter * 4 + h * 2) * MODELS
                        nc.vector.tensor_tensor(
                            out=ot[:, o0:o0 + 2 * MODELS]
                                .rearrange("p (g m) -> p g m", g=2),
                            in0=y_sb[:].unsqueeze(1).to_broadcast([P, 2, MODELS]),
                            in1=pp[:].rearrange("p (g m) -> p g m", g=2),
                            op=mybir.AluOpType.add,
                        )
                if c == NB - 1:
                    emit_store(c, go, ot)
                else:
                    pending_stores.append((c, go, ot))

    nc.compile()
    _MODULE_CACHE["nc"] = nc
    return nc


def make_in_maps(tokens, emb_table, W, b):
    import ml_dtypes

    tokens = np.ascontiguousarray(np.asarray(tokens, dtype=np.int32))
    emb_table = np.ascontiguousarray(np.asarray(emb_table, dtype=np.float32))
    W = np.asarray(W, dtype=np.float32)
    b = np.asarray(b, dtype=np.float32)

    wp = np.ascontiguousarray(
        (W * np.float32(np.sqrt(np.float32(MODELS)))).astype(ml_dtypes.bfloat16)
    )
    peb = np.ascontiguousarray(
        (_positional_encoding(SEQ, MODELS) + b[None, :].astype(np.float32))
        .astype(ml_dtypes.bfloat16)
    )
    ident = np.eye(P, dtype=np.float32)
    ones = np.ones((1, P), dtype=ml_dtypes.bfloat16)

    in_maps = []
    for c in range(N_CORES):
        in_maps.append({
            "tok": tokens[c * BS:(c + 1) * BS],
            "emb": emb_table,
            "w": wp,
            "pe": peb,
            "ident": ident,
            "ones": ones,
        })
    return in_maps


def run(tokens, emb_table, W, b, trace=False):
    """Run on 8 NeuronCores; returns (full_output, BassKernelResults)."""
    from concourse import bass_utils

    nc = build_module()
    in_maps = make_in_maps(tokens, emb_table, W, b)
    res = bass_utils.run_bass_kernel_spmd(
        nc, in_maps, core_ids=list(range(N_CORES)), trace=trace
    )
    outs = [r["out"] for r in res.results]
    full = np.concatenate(outs, axis=0)
    return full, res


def kernel(tokens, emb_table, W, b):
    full, _ = run(tokens, emb_table, W, b, trace=False)
    return full


# revision 11
# speedup vs baseline: 2.1037x; 1.2474x over previous
"""Trainium2 Bass kernel for nn_EmbeddingLayer (embedding lookup + dense + positional encoding).

Computation (per reference):
    x = emb_table[tokens] * sqrt(512)          [B, F, E]
    x = x.reshape(B, F*E) @ W + b              [B, M]
    out = x[:, None, :] + pe[:128, :]          [B, S, M]   (1 GiB f32 output)

Strategy: data-parallel over batch across 8 cores (512 rows each); the
74 MB table and the dense weight are replicated.  Per core:
  - indirect-DMA gather, one token column per instruction (HW honors one
    index per partition), 256 gathers of 128 rows x 128 B
  - PE transpose (f32 fast transpose mode) -> xT cast to bf16, bf16
    matmul against pre-scaled bf16 W (scale/bias folded on host)
  - pe rows broadcast across partitions with a ones(1x128) bf16 matmul
    on PE; the f32 psum result is consumed directly by the DVE add
    (no psum->SBUF copy)
  - DVE broadcast-add y[b,m] + pe[s,m] -> f32 out tiles
  - 1 MiB HWDGE stores of the 128 MiB per-core output (the roofline)
Loop order streams per batch-chunk so adds/stores begin as soon as the
first chunk's gathers land.
"""

import sys

import numpy as np

if "/opt/trn_rl_repo" not in sys.path:
    sys.path.insert(0, "/opt/trn_rl_repo")

BATCH = 4096
FEATURES = 64
VOCAB = 580000
EMB = 32
MODELS = 512
SEQ = 128
N_CORES = 8
BS = BATCH // N_CORES  # 512 rows per core

P = 128                 # partitions
NB = BS // P            # 4 batch chunks per core
KC = (FEATURES * EMB) // P  # 16 k-chunks of 128

_MODULE_CACHE = {}


def _positional_encoding(position, d_model):
    # mirror of reference._positional_encoding, in numpy f32
    pos = np.arange(position, dtype=np.float32)[:, None]
    i = np.arange(d_model, dtype=np.float32)[None, :]
    angle_rates = 1.0 / np.power(
        10000.0, (2.0 * np.floor(i / 2.0)) / np.float32(d_model)
    )
    angles = (pos * angle_rates).astype(np.float32)
    even = (np.arange(d_model) % 2 == 0)[None, :]
    pe = np.where(even, np.sin(angles), np.cos(angles)).astype(np.float32)
    return pe  # [S, M]


def build_module():
    """Build + compile the per-core Bass module (identical program on all cores)."""
    if "nc" in _MODULE_CACHE:
        return _MODULE_CACHE["nc"]

    from contextlib import ExitStack

    import concourse.bass as bass
    import concourse.tile as tile
    from concourse import bacc, mybir

    f32 = mybir.dt.float32
    bf16 = mybir.dt.bfloat16
    i32 = mybir.dt.int32

    nc = bacc.Bacc("TRN2", target_bir_lowering=False, debug=False,
                   num_devices=N_CORES)

    tok = nc.dram_tensor("tok", [BS, FEATURES], i32, kind="ExternalInput").ap()
    emb = nc.dram_tensor("emb", [VOCAB, EMB], f32, kind="ExternalInput").ap()
    w = nc.dram_tensor("w", [FEATURES * EMB, MODELS], bf16, kind="ExternalInput").ap()
    pe = nc.dram_tensor("pe", [SEQ, MODELS], bf16, kind="ExternalInput").ap()
    ident = nc.dram_tensor("ident", [P, P], f32, kind="ExternalInput").ap()
    ones = nc.dram_tensor("ones", [1, P], bf16, kind="ExternalInput").ap()
    out = nc.dram_tensor("out", [BS, SEQ, MODELS], f32, kind="ExternalOutput").ap()

    with tile.TileContext(nc) as tc, ExitStack() as ctx:
        const = ctx.enter_context(tc.tile_pool(name="const", bufs=1))
        tok_pool = ctx.enter_context(tc.tile_pool(name="tok", bufs=NB))
        x_pool = ctx.enter_context(tc.tile_pool(name="x", bufs=2))
        xT_pool = ctx.enter_context(tc.tile_pool(name="xT", bufs=2))
        y_pool = ctx.enter_context(tc.tile_pool(name="y", bufs=2))
        perow_pool = ctx.enter_context(tc.tile_pool(name="perow", bufs=8))
        out_pool = ctx.enter_context(tc.tile_pool(name="outp", bufs=6))
        psum_t = ctx.enter_context(tc.tile_pool(name="pst", bufs=1, space="PSUM"))
        psum_y = ctx.enter_context(tc.tile_pool(name="psy", bufs=1, space="PSUM"))
        psum_p = ctx.enter_context(tc.tile_pool(name="psp", bufs=3, space="PSUM"))

        # constants — all input loads go on the ScalarE HWDGE ring so the
        # Sync ring carries only the big output stores
        w_sb = const.tile([P, KC * MODELS], bf16)
        nc.scalar.dma_start(
            w_sb[:].rearrange("p (kc m) -> p kc m", kc=KC),
            w.rearrange("(kc p) m -> p kc m", p=P),
        )
        id_sb = const.tile([P, P], f32)
        nc.scalar.dma_start(id_sb[:], ident[:])
        ones_sb = const.tile([1, P], bf16)
        nc.scalar.dma_start(ones_sb[:], ones[:])

        tok_sbs = []
        for c in range(NB):
            tok_sb = tok_pool.tile([P, FEATURES], i32)
            nc.scalar.dma_start(tok_sb[:], tok[c * P:(c + 1) * P, :])
            tok_sbs.append(tok_sb)

        # Software-pipelined windows: window w gathers chunk w (if any) while
        # adding/storing chunk w-1.  Emission is interleaved at ~10 us
        # granularity so every engine's in-order stream matches real-time
        # data availability.  Stores ride the same SWDGE queue as the
        # gathers (HWDGE queues starve while SWDGE traffic flows).
        y_sbs = {}

        def emit_gather_half(c, step, h, x_sb, tok_sb):
            # 8 gathers = 2 k-chunks
            for kc in (4 * step + 2 * h, 4 * step + 2 * h + 1):
                for jf in range(4):
                    f = 4 * kc + jf
                    nc.gpsimd.indirect_dma_start(
                        out=x_sb[:, f * EMB:(f + 1) * EMB],
                        out_offset=None,
                        in_=emb[:],
                        in_offset=bass.IndirectOffsetOnAxis(
                            ap=tok_sb[:, f:f + 1], axis=0
                        ),
                    )

        def emit_transpose_half(step, h, x_sb, pt):
            for jj in range(2):
                kc = 4 * step + 2 * h + jj
                nc.tensor.transpose(
                    out=pt[:, (2 * h + jj) * P:(2 * h + jj + 1) * P],
                    in_=x_sb[:, kc * P:(kc + 1) * P],
                    identity=id_sb[:],
                )

        for w in range(NB + 1):
            gc = w if w < NB else None      # chunk being gathered
            ac = w - 1 if w >= 1 else None  # chunk being added/stored
            if gc is not None:
                x_sb = x_pool.tile([P, FEATURES * EMB], f32)
                xT_sb = xT_pool.tile([P, KC * P], bf16)
                tok_sb = tok_sbs[gc]
            y_sb = y_sbs.get(ac)

            for step in range(4):
                # prefetch pe rows for this step's 32 seq positions (ACT ring)
                perows = []
                if ac is not None:
                    for q in range(8):
                        s0 = step * 32 + q * 4
                        perow = perow_pool.tile([1, 4 * MODELS], bf16)
                        nc.scalar.dma_start(
                            perow[:].rearrange("p (g m) -> p g m", g=4),
                            pe[s0:s0 + 4, :].unsqueeze(0),
                        )
                        perows.append(perow)

                if gc is not None:
                    pt = psum_t.tile([P, 4 * P], f32)
                for h in range(2):          # one 'go' output tile per half-step
                    if gc is not None:
                        emit_gather_half(gc, step, h, x_sb, tok_sb)
                    if ac is not None:
                        go = step * 2 + h
                        ot = out_pool.tile([P, 16 * MODELS], bf16)
                        for pq in range(8):  # 8 pp tiles x 2 seq rows
                            perow = perows[h * 4 + pq // 2]
                            pp = psum_p.tile([P, 2 * MODELS], f32)
                            for u in range(2):
                                r0 = ((pq % 2) * 2 + u) * MODELS
                                nc.tensor.matmul(
                                    pp[:, u * MODELS:(u + 1) * MODELS],
                                    lhsT=ones_sb[:],
                                    rhs=perow[:, r0:r0 + MODELS],
                                    start=True,
                                    stop=True,
                                )
                            o0 = pq * 2 * MODELS
                            nc.vector.tensor_tensor(
                                out=ot[:, o0:o0 + 2 * MODELS]
                                    .rearrange("p (g m) -> p g m", g=2),
                                in0=y_sb[:].unsqueeze(1)
                                    .to_broadcast([P, 2, MODELS]),
                                in1=pp[:].rearrange("p (g m) -> p g m", g=2),
                                op=mybir.AluOpType.add,
                            )
                        s0 = go * 16
                        nc.gpsimd.dma_start(
                            out[ac * P:(ac + 1) * P, s0:s0 + 16, :],
                            ot[:].rearrange("p (g m) -> p g m", g=16),
                        )
                    if gc is not None:
                        emit_transpose_half(step, h, x_sb, pt)
                if gc is not None:
                    nc.vector.tensor_copy(
                        xT_sb[:, step * 4 * P:(step + 1) * 4 * P], pt[:]
                    )

            if gc is not None:
                py = psum_y.tile([P, MODELS], f32)
                for kc in range(KC):
                    nc.tensor.matmul(
                        py[:],
                        lhsT=xT_sb[:, kc * P:(kc + 1) * P],
                        rhs=w_sb[:, kc * MODELS:(kc + 1) * MODELS],
                        start=(kc == 0),
                        stop=(kc == KC - 1),
                    )
                y_new = y_pool.tile([P, MODELS], f32)
                nc.scalar.copy(y_new[:], py[:])
                y_sbs[gc] = y_new

    nc.compile()
    _MODULE_CACHE["nc"] = nc
    return nc


def make_in_maps(tokens, emb_table, W, b):
    import ml_dtypes

    tokens = np.ascontiguousarray(np.asarray(tokens, dtype=np.int32))
    emb_table = np.ascontiguousarray(np.asarray(emb_table, dtype=np.float32))
    W = np.asarray(W, dtype=np.float32)
    b = np.asarray(b, dtype=np.float32)

    wp = np.ascontiguousarray(
        (W * np.float32(np.sqrt(np.float32(MODELS)))).astype(ml_dtypes.bfloat16)
    )
    peb = np.ascontiguousarray(
        (_positional_encoding(SEQ, MODELS) + b[None, :].astype(np.float32))
        .astype(ml_dtypes.bfloat16)
    )
    ident = np.eye(P, dtype=np.float32)
    ones = np.ones((1, P), dtype=ml_dtypes.bfloat16)

    in_maps = []
    for c in range(N_CORES):
        in_maps.append({
            "tok": tokens[c * BS:(c + 1) * BS],
            "emb": emb_table,
            "w": wp,
            "pe": peb,
            "ident": ident,
            "ones": ones,
        })
    return in_maps


def run(tokens, emb_table, W, b, trace=False):
    """Run on 8 NeuronCores; returns (full_output, BassKernelResults)."""
    from concourse import bass_utils

    nc = build_module()
    in_maps = make_in_maps(tokens, emb_table, W, b)
    res = bass_utils.run_bass_kernel_spmd(
        nc, in_maps, core_ids=list(range(N_CORES)), trace=trace
    )
    outs = [r["out"] for r in res.results]
    full = np.concatenate(outs, axis=0)
    return full, res


def kernel(tokens, emb_table, W, b):
    full, _ = run(tokens, emb_table, W, b, trace=False)
    return full
